# revision 31
# baseline (speedup 1.0000x reference)
"""MultiHeadMixer Trainium2 kernel.

Math (matches the reference):
  x: (B=8, E=1024, S=2048), weight: (H=16, S), bias: (H, S)
  xg = x.reshape(B*H, D, S).reshape(H, B, D, S)          # D = E//H = 64
  out[m, r, t] = sum_{s<=t} xg[m, r//D, r%D, s] * weight[m, t-s] + bias[m, t]
  return out.reshape(B, E, S)

Per head this is C = A @ M with M the upper-triangular Toeplitz matrix
M[s, t] = w[t-s].  Key structure: every 128x128 block of M depends only on
diff = t_tile - s_tile, and all such blocks are column slices of one
(128, S) "shifted weight" array T with T[i, c] = w[c - i] (0 for c < i).

Device computation (per head): C^T[t, r] = sum_s M[s, t] * A[r, s], i.e.
  psum[tb] += T[:, d*128:(d+1)*128].T @ xT[k]      for d = tb - k >= 0
with xT = A^T staged per 128-row s-tile.  Stationary operand = Toeplitz
block (reused across all k for fixed d), moving operand = xT tile (N=512).
Lower-triangle blocks are skipped entirely (136 of 256 matmuls per head).

Sharding: head-parallel, 2 heads per core across 8 cores; weight/bias/x
slices per head, no collectives.
"""

import numpy as np

import concourse.bass as bass
import concourse.mybir as mybir
import concourse.tile as tile
from concourse import bacc
from concourse.bass_utils import run_bass_kernel_spmd

B, E, S, H = 8, 1024, 2048, 16
D = E // H            # 64   per-head hidden dim
NCORES = 8
HPC = H // NCORES     # 2    heads per core
R = B * D             # 512  rows per head (moving free dim)
KT = S // 128         # 16   128-wide tiles along the sequence axis

import os as _os
_MM_DTYPE = _os.environ.get("MM_DTYPE", "float16")
if _MM_DTYPE == "bfloat16":
    import ml_dtypes
    DT = mybir.dt.bfloat16
    NPDT = ml_dtypes.bfloat16
elif _MM_DTYPE == "float16":
    DT = mybir.dt.float16
    NPDT = np.float16
else:
    DT = mybir.dt.float32r  # fp32 bits, fast PE mode
    NPDT = np.float32
F32 = mybir.dt.float32

_CACHED_NC = None


def _ensure_axon_hooks_stub():
    # bass_utils' axon trace path does `from antenv.axon_hooks import ...`;
    # the agent image's antenv lacks that module.  Provide a stub so a
    # BASS_TRACE=1 environment degrades to "no trace" instead of crashing.
    try:
        import antenv.axon_hooks  # noqa: F401
    except ImportError:
        import sys
        import types
        import antenv
        mod = types.ModuleType("antenv.axon_hooks")
        mod._hook = None
        mod.set_axon_ntff_profile_hook = lambda h: setattr(mod, "_hook", h)
        mod.get_axon_ntff_profile_hook = lambda: mod._hook
        sys.modules["antenv.axon_hooks"] = mod
        antenv.axon_hooks = mod


_ensure_axon_hooks_stub()


GRP = 4                      # output tiles per PSUM group (4 banks per head)


def _build_bass():
    # Constraint honored throughout: a Matmult can carry at most ONE sync
    # wait in walrus codegen.  So every matmul is arranged to have at most
    # one not-yet-synced dependency at its point in PE program order:
    #  * `comb` packs the Toeplitz array and the k=0 x-tile into one DMA,
    #    so each head's first matmul waits on exactly that DMA;
    #  * the two heads use disjoint 4-bank PSUM pools, so each head's
    #    first group gets fresh banks (no bank-free wait);
    #  * later groups recycle banks; their start matmuls use k=0 (already
    #    synced) with d=tb weights, leaving only the bank-free wait.
    nc = bacc.Bacc()
    xt = nc.dram_tensor("xt", [HPC, S, R], DT, kind="ExternalInput")
    comb = nc.dram_tensor("comb", [HPC, 128, S + R], DT, kind="ExternalInput")
    biast = nc.dram_tensor("biast", [HPC, 128, KT], F32, kind="ExternalInput")
    out = nc.dram_tensor("out", [HPC, KT // GRP, 128, GRP * R], F32,
                         kind="ExternalOutput")

    with tile.TileContext(nc) as tc:
        with (
            tc.tile_pool(name="xp", bufs=HPC * (KT - 1)) as xp,
            tc.tile_pool(name="tp", bufs=HPC) as tp,
            tc.tile_pool(name="bp", bufs=HPC) as bp,
            tc.tile_pool(name="op", bufs=HPC * (KT // GRP)) as op,
            tc.tile_pool(name="sp", bufs=1) as sp,
            tc.tile_pool(name="psA", bufs=GRP, space="PSUM") as psA,
            tc.tile_pool(name="psB", bufs=GRP, space="PSUM") as psB,
        ):
            # PE warm-up: the HAM clock gate needs ~3.4us of sustained PE
            # activity to lift the 1.2GHz -> 2.4GHz throttle, and the PE
            # sits idle for ~11us anyway while the first input DMA lands.
            # Zero matmuls on a scratch tile warm the clock for free.
            warm = sp.tile([128, R + 128], DT, tag="warm", name="warm")
            nc.gpsimd.memset(warm[:], 0)
            wps = psA.tile([128, R], F32, tag="acc", name="warm_ps")
            NWARM = 7
            for i in range(NWARM):
                nc.tensor.matmul(wps[:], warm[:, R:R + 128], warm[:, 0:R],
                                 start=(i == 0), stop=(i == NWARM - 1))

            # DMA emission follows consumption order: per head comb (first
            # matmul) + bias (first PSUM-drain copy) + early x tiles, then
            # later x tiles interleaved by group.
            combs, biases, xs = [], [], []

            def load_x(h, k):
                xk = xp.tile([128, R], DT, tag="x", name=f"x{h}_{k}")
                nc.sync.dma_start(xk[:], xt[h, k * 128:(k + 1) * 128, :])
                xs[h][k] = xk

            # comb column layout: [Td0..Td3 | x0 | Td4..Td15]; the first
            # 1024 columns are a separate small DMA so the head's start
            # matmuls (which touch only Td0..3 + x0) begin ~1.5us sooner.
            CA = GRP * 128 + R          # comb_a width
            for h in range(HPC):
                c_sb = tp.tile([128, S + R], DT, tag="T", name=f"comb{h}")
                nc.sync.dma_start(c_sb[:, :CA], comb[h, :, :CA])
                nc.sync.dma_start(c_sb[:, CA:], comb[h, :, CA:])
                combs.append(c_sb)
                xs.append({0: c_sb[:, GRP * 128:CA]})
                b_sb = bp.tile([128, KT], F32, tag="bias", name=f"bias{h}")
                nc.sync.dma_start(b_sb[:], biast[h])
                biases.append(b_sb)
                # Absorb the bias-DMA wait on DVE so the PSUM-drain copies
                # below only ever wait on the PE semaphore (walrus allows a
                # single sync wait per compute instruction).
                bscr = sp.tile([128, KT], F32, tag=f"bscr{h}", name=f"bscr{h}")
                nc.vector.tensor_copy(bscr[:], b_sb[:])
                for k in range(1, GRP):
                    load_x(h, k)
            for h in range(HPC):
                for k in range(GRP, 2 * GRP):
                    load_x(h, k)
            for kg in range(2, KT // GRP + 1):
                for h in range(HPC):
                    for k in range(kg * GRP, min(KT, (kg + 1) * GRP)):
                        load_x(h, k)

            def w(h, d):        # Toeplitz weight block for offset d
                c = d * 128 if d < GRP else R + d * 128
                return combs[h][:, c:c + 128]

            for g in range(KT // GRP):
                tbs = range(GRP * g, GRP * (g + 1))
                for h in range(HPC):
                    pool = psA if h == 0 else psB
                    ps = {}
                    for tb in tbs:
                        ps[tb] = pool.tile([128, R], F32, tag="acc",
                                           name=f"acc{h}_{tb}")
                    # Start every tile's accumulation at k=0 (d=tb): for g=0
                    # that is gated only on the small comb_a DMA, for g>0 it
                    # leaves the bank-free semaphore as the only sync wait.
                    for tb in tbs:
                        nc.tensor.matmul(ps[tb][:], w(h, tb), xs[h][0],
                                         start=True, stop=(tb == 0))
                    for d in range(tbs.stop - 1):
                        for tb in range(max(tbs.start, d + 1), tbs.stop):
                            nc.tensor.matmul(
                                ps[tb][:], w(h, d), xs[h][tb - d],
                                start=False, stop=(d == tb - 1))
                    # Merged staging tiles: big output DMAs amortize issue
                    # cost; the split lets earlier pieces stream out while
                    # later output tiles are still accumulating.  The very
                    # last piece is kept small to shorten the tail chain.
                    last = (h == HPC - 1 and g == KT // GRP - 1)
                    pieces = ((0, 2), (2, 3), (3, 4)) if last else ((0, 2), (2, 4))
                    for p0, p1 in pieces:
                        w_ = p1 - p0
                        o = op.tile([128, w_ * R], F32, tag="o",
                                    name=f"o{h}_{g}_{p0}")
                        for j, tb in enumerate(tbs[p0:p1]):
                            nc.vector.tensor_scalar_add(
                                o[:, j * R:(j + 1) * R], ps[tb][:],
                                biases[h][:, tb:tb + 1])
                        nc.sync.dma_start(
                            out[h, g][:, p0 * R:p1 * R], o[:])
    nc.compile()
    return nc


def _get_nc():
    global _CACHED_NC
    if _CACHED_NC is None:
        _CACHED_NC = _build_bass()
    return _CACHED_NC


def _toeplitz_rows(w_row):
    """(S,) weight -> (128, S) array T, T[i, c] = w[c-i] (0 where c < i)."""
    wpad = np.concatenate([np.zeros(127, np.float32),
                           np.asarray(w_row, np.float32)])
    sw = np.lib.stride_tricks.sliding_window_view(wpad, S)   # (128, S)
    return sw[127::-1]


def run(x, weight, bias, trace=False, trace_kwargs=None, trace_cores=None):
    x = np.ascontiguousarray(np.asarray(x, np.float32))
    weight = np.asarray(weight, np.float32)
    bias = np.asarray(bias, np.float32)

    xg = x.reshape(B * H, D, S).reshape(H, B, D, S)   # view, no copy

    in_maps = []
    for c in range(NCORES):
        xt = np.empty((HPC, S, R), NPDT)
        comb = np.empty((HPC, 128, S + R), NPDT)
        biast = np.empty((HPC, 128, KT), np.float32)
        for i in range(HPC):
            m = HPC * c + i
            xt[i] = xg[m].reshape(R, S).T
            tw = _toeplitz_rows(weight[m])
            # comb columns: [Td0..Td3 | x0 | Td4..Td15]
            comb[i, :, :GRP * 128] = tw[:, :GRP * 128]
            comb[i, :, GRP * 128:GRP * 128 + R] = xt[i, :128, :]
            comb[i, :, GRP * 128 + R:] = tw[:, GRP * 128:]
            biast[i] = bias[m].reshape(KT, 128).T
        in_maps.append({"xt": xt, "comb": comb, "biast": biast})

    nc = _get_nc()
    kw = {}
    if trace:
        kw["trace"] = True
        if trace_kwargs:
            kw["trace_kwargs"] = trace_kwargs
        if trace_cores is not None:
            kw["trace_cores"] = trace_cores
    res = run_bass_kernel_spmd(nc, in_maps, core_ids=list(range(NCORES)), **kw)

    outg = np.empty((H, B, D, S), np.float32)
    for c in range(NCORES):
        o = res.results[c]["out"]                 # (HPC, KT//GRP, 128, GRP*R)
        o = o.reshape(HPC, KT // GRP, 128, GRP, R)
        for i in range(HPC):
            m = HPC * c + i
            ct = o[i].transpose(0, 2, 1, 3).reshape(S, R)   # (t, r)
            outg[m] = ct.T.reshape(B, D, S)
    return outg.reshape(B, E, S), res


def kernel(x, weight, bias):
    out, _ = run(x, weight, bias, trace=False)
    return out
